# revision 50
# baseline (speedup 1.0000x reference)
"""Trainium2 Bass kernel for nn_DiffusionModel (auction-matched flow targets).

Self-contained: accepts FULL inputs (cloud [16,2048,3], noise [16,2048,3],
t [16]), shards batch over 8 NeuronCores (2 samples per core), runs the full
5-iteration bijective auction per sample on device, returns [2,16,2048,3].

Device algorithm per sample (faithful dense auction, f32):
  x0 = cloud / std(cloud, ddof=1)
  V0[i,j] = -(|n_i|^2 + |x0_j|^2 - 2 n_i.x0_j)   (PE K=4 fp32 matmul + ACT)
  5x: Vp = V0 - price[j] (broadcast); per-row top2 + argmax (DVE max/max_index);
      bid = m1 - m2 + eps; scatter-max via per-partition dedup + local_scatter of
      f32 bid bit-planes (2xu16) + partition_all_reduce(max) + lexicographic lo
      select; price replaced at targeted columns (exact replace semantics).
  output: x0_aligned positional extraction (iota==jstar mask, mul+reduce per
  coord), x_t = (1-t) x0a + t noise, v = noise - x0a.
"""
import numpy as np

P = 128
N = 2048
NG = 16          # row groups per sample (NG * P = N rows)
D = 3
SPC = 2          # samples per core
EPS = 1e-3
NCORES = 8
REPEAT = 1       # benchmark knob: repeat the whole per-core pipeline


def _build_program():
    import concourse.bass as bass
    import concourse.tile as tile
    from concourse import bacc, mybir, bass_isa

    fp32 = mybir.dt.float32
    u16 = mybir.dt.uint16
    i16 = mybir.dt.int16
    i32 = mybir.dt.int32
    OP = mybir.AluOpType
    AX = mybir.AxisListType
    AF = mybir.ActivationFunctionType

    nc = bacc.Bacc("TRN2", target_bir_lowering=False, debug=False,
                   enable_asserts=False)

    # ---- DRAM I/O ----
    noiseT_d = nc.dram_tensor("noiseT", [SPC, 3, N], fp32, kind="ExternalInput")
    cloudT_d = nc.dram_tensor("cloudT", [SPC, 3, N], fp32, kind="ExternalInput")
    cloudR_d = nc.dram_tensor("cloudR", [SPC, P, NG * D], fp32, kind="ExternalInput")
    noiseR_d = nc.dram_tensor("noiseR", [SPC, P, NG * D], fp32, kind="ExternalInput")
    tv_d = nc.dram_tensor("tv", [SPC, 1], fp32, kind="ExternalInput")
    ltc_d = nc.dram_tensor("ltc", [P, NG * NG], u16, kind="ExternalInput")
    ecs_d = nc.dram_tensor("ecs", [35, 3 * P], fp32, kind="ExternalInput")
    onesrow_d = nc.dram_tensor("onesrow", [1, N], fp32, kind="ExternalInput")
    out_d = nc.dram_tensor("out", [SPC, 2, P, NG * D], fp32, kind="ExternalOutput")
    import os
    DBG = bool(int(os.environ.get("BASSDBG", "0")))
    if DBG:
        dbg_d = nc.dram_tensor("dbg", [8, P, N], fp32, kind="ExternalOutput")
        dbgu_d = nc.dram_tensor("dbgu", [4, P, N], u16, kind="ExternalOutput")
        dbgv_d = nc.dram_tensor("dbgv", [NG, P, N], fp32, kind="ExternalOutput")
        dbgit_d = nc.dram_tensor("dbgit", [5, 4, P, N], fp32, kind="ExternalOutput")

    with tile.TileContext(nc) as tc:
        with (
            tc.tile_pool(name="v0", bufs=1) as v0pool,
            tc.tile_pool(name="dense", bufs=1) as dp,
            tc.tile_pool(name="small", bufs=1) as sp,
            tc.tile_pool(name="psA", bufs=4, space="PSUM") as psA,
            tc.tile_pool(name="psB", bufs=2, space="PSUM") as psB,
        ):
            # ---- constants (persist across both samples) ----
            LTC = sp.tile([P, NG * NG], u16, tag="ltc")
            nc.sync.dma_start(LTC[:], ltc_d.ap())
            ones_128x1 = sp.tile([P, 1], fp32, tag="ones_128x1")
            onescol = sp.tile([67, 1], fp32, tag="onescol")
            MINUS1 = sp.tile([P, NG], fp32, tag="minus1")
            ZU16 = sp.tile([P, 1], u16, tag="zu16")
            # coordinate-selector weights at base partition 32 (K=3 matmuls)
            ECS = sp.tile([35, 3 * P], fp32, tag="ecs")
            nc.sync.dma_start(ECS[:], ecs_d.ap())
            nc.vector.memset(ones_128x1[:], 1.0)
            nc.vector.memset(onescol[:], 0.0)
            nc.vector.memset(onescol[64:67, :], 1.0)
            nc.vector.memset(MINUS1[:], -1.0)
            nc.vector.memset(ZU16[:], 0)

            for s in [s for _ in range(REPEAT) for s in range(SPC)]:
                # ================= prep =================
                # strip1 rows 32-35: lhsT (nx,ny,nz,1); rows 64-66: cloudT^2
                # strip2 rows 32-35: rhs (-2*x0 coords, yn)
                strip1 = sp.tile([P, N], fp32, tag="strip1")
                strip2 = sp.tile([P, N], fp32, tag="strip2")
                nT = strip1[32:36, :]
                csq = strip1[64:67, :]
                X04 = strip2[32:36, :]
                cR = sp.tile([P, NG * D], fp32, tag="cR")
                nR = sp.tile([P, NG * D], fp32, tag="nR")
                nc.sync.dma_start(nT[0:3, :], noiseT_d.ap()[s])
                nc.sync.dma_start(nT[3:4, :], onesrow_d.ap())
                nc.sync.dma_start(csq[:], cloudT_d.ap()[s])
                nc.sync.dma_start(X04[0:3, :], cloudT_d.ap()[s])
                nc.sync.dma_start(cR[:], cloudR_d.ap()[s])
                nc.sync.dma_start(nR[:], noiseR_d.ap()[s])

                # ---- std (two-pass, ddof=1) ----
                red = sp.tile([P, 1], fp32, tag="red")
                nc.vector.tensor_reduce(red[:], cR[:], axis=AX.X, op=OP.add)
                pm = psB.tile([1, 1], fp32, tag="pm")
                nc.tensor.matmul(pm[:], red[:], ones_128x1[:])
                negmean = sp.tile([1, 1], fp32, tag="negmean")
                nc.scalar.activation(negmean[:], pm[:], AF.Identity,
                                     bias=0.0, scale=-1.0 / (N * D))
                negmeanb = sp.tile([P, 1], fp32, tag="negmeanb")
                nc.gpsimd.partition_broadcast(negmeanb[:], negmean[:], channels=P)
                sqdev = sp.tile([P, NG * D], fp32, tag="sqdev")
                nc.scalar.activation(sqdev[:], cR[:], AF.Square,
                                     bias=negmeanb[:], scale=1.0)
                nc.vector.tensor_reduce(red[:], sqdev[:], axis=AX.X, op=OP.add)
                pv = psB.tile([1, 1], fp32, tag="pm")
                nc.tensor.matmul(pv[:], red[:], ones_128x1[:])
                var1 = sp.tile([1, 1], fp32, tag="var1")
                nc.scalar.activation(var1[:], pv[:], AF.Identity,
                                     bias=0.0, scale=1.0 / (N * D - 1))
                std1 = sp.tile([1, 1], fp32, tag="std1")
                nc.scalar.activation(std1[:], var1[:], AF.Sqrt,
                                     bias=0.0, scale=1.0)
                invvar = sp.tile([1, 1], fp32, tag="invvar")
                nc.vector.reciprocal(invvar[:], var1[:])
                invstd = sp.tile([1, 1], fp32, tag="invstd")
                nc.vector.reciprocal(invstd[:], std1[:])
                stdb = sp.tile([P, 1], fp32, tag="stdb")
                nc.gpsimd.partition_broadcast(stdb[:], invstd[:], channels=P)

                # ---- X04 coords = (cloudT / std) * -2 ; csq = cloudT^2 ----
                nc.vector.tensor_scalar(X04[0:3, :], X04[0:3, :],
                                        stdb[32:35, :], -2.0,
                                        op0=OP.mult, op1=OP.mult)
                nc.scalar.activation(csq[:], csq[:], AF.Square,
                                     bias=0.0, scale=1.0)
                # yn row: sum(cloudT^2) * (1/var)
                for tcol in range(4):
                    pyn = psB.tile([1, 512], fp32, tag="pyn")
                    nc.tensor.matmul(pyn[:], onescol[64:67, :],
                                     csq[:, 512 * tcol:512 * (tcol + 1)])
                    ynsb = dp.tile([1, 512], fp32, tag="eqd")
                    nc.scalar.activation(ynsb[:], pyn[:], AF.Identity,
                                         bias=0.0, scale=invvar[:])
                    nc.sync.dma_start(X04[3:4, 512 * tcol:512 * (tcol + 1)],
                                      ynsb[:])

                # ---- xn per row-group ----
                nsq = sp.tile([P, NG * D], fp32, tag="sqdev")
                nc.scalar.activation(nsq[:], nR[:], AF.Square, bias=0.0, scale=1.0)
                xn = sp.tile([P, NG], fp32, tag="xn")
                nc.vector.tensor_reduce(
                    xn[:], nsq[:].rearrange("p (g d) -> p g d", d=D),
                    axis=AX.X, op=OP.add)
                negxn = sp.tile([P, NG], fp32, tag="negxn")
                nc.vector.tensor_scalar(negxn[:], xn[:], -1.0, None, op0=OP.mult)

                # ---- V0 = -(dist) ----
                # ---- V0 build fused with the iteration-0 scan so PE/ACT
                # (matmuls) pipeline with DVE (max/max_index) per group ----
                V0 = v0pool.tile([P, NG * N], fp32, tag="v0all")
                TOP8 = sp.tile([P, NG * 8], fp32, tag="top8")
                IDX8 = sp.tile([P, NG * 8], u16, tag="idx8")
                for g in range(NG):
                    for tcol in range(4):
                        ps = psA.tile([P, 512], fp32, tag="ps")
                        nc.tensor.matmul(ps[:], nT[:, P * g:P * (g + 1)],
                                         X04[:, 512 * tcol:512 * (tcol + 1)])
                        nc.scalar.activation(
                            V0[:, g * N + 512 * tcol: g * N + 512 * (tcol + 1)],
                            ps[:], AF.Identity, bias=negxn[:, g:g + 1], scale=-1.0)
                    v0g = V0[:, g * N:(g + 1) * N]
                    nc.vector.max(TOP8[:, 8 * g:8 * (g + 1)], v0g)
                    nc.vector.max_index(IDX8[:, 8 * g:8 * (g + 1)],
                                        TOP8[:, 8 * g:8 * (g + 1)], v0g)

                # ================= auction iterations =================
                PBC = dp.tile([P, N], fp32, tag="pbc")
                nc.vector.memset(PBC[:], 0.0)
                BIDF = sp.tile([P, NG], fp32, tag="bidf")
                JF = sp.tile([P, NG], fp32, tag="jf")

                for it in range(5):
                    last = (it == 4)
                    if it > 0:
                        for g in range(NG):
                            v0g = V0[:, g * N:(g + 1) * N]
                            Vp = dp.tile([P, N], fp32, tag="vps", bufs=3)
                            mode = os.environ.get("SUBENG", "dve")
                            if mode == "gp" or (mode == "split" and g >= 10):
                                sub_eng = nc.gpsimd
                            else:
                                sub_eng = nc.vector
                            sub_eng.tensor_tensor(Vp[:], v0g, PBC[:],
                                                  op=OP.subtract)
                            nc.vector.max(TOP8[:, 8 * g:8 * (g + 1)], Vp[:])
                            nc.vector.max_index(IDX8[:, 8 * g:8 * (g + 1)],
                                                TOP8[:, 8 * g:8 * (g + 1)],
                                                Vp[:])

                    t8v = TOP8[:].rearrange("p (g k) -> p g k", k=8)
                    m1 = t8v[:, :, 0]
                    m2 = t8v[:, :, 1]
                    jsel = IDX8[:].rearrange("p (g k) -> p g k", k=8)[:, :, 0]
                    nc.vector.tensor_copy(JF[:], jsel)
                    if DBG and s == 0:
                        nc.sync.dma_start(dbgit_d.ap()[it, 0][:, 0:NG * 8],
                                          TOP8[:])
                        nc.sync.dma_start(dbgit_d.ap()[it, 1][:, 0:NG], JF[:])
                    if last:
                        break
                    nc.vector.tensor_tensor(BIDF[:], m1, m2, op=OP.subtract)
                    nc.vector.tensor_scalar(BIDF[:], BIDF[:], float(EPS), None,
                                            op0=OP.add)

                    # ---- dedup within partition (16 bids each) ----
                    ja = JF[:].unsqueeze(2).broadcast_to([P, NG, NG])
                    jb = JF[:].unsqueeze(1).broadcast_to([P, NG, NG])
                    ba = BIDF[:].unsqueeze(2).broadcast_to([P, NG, NG])
                    bb = BIDF[:].unsqueeze(1).broadcast_to([P, NG, NG])
                    dA = sp.tile([P, NG * NG], u16, tag="dA")
                    dB = sp.tile([P, NG * NG], u16, tag="dB")
                    dC = sp.tile([P, NG * NG], u16, tag="dC")
                    dAv = dA[:].rearrange("p (a b) -> p a b", b=NG)
                    dBv = dB[:].rearrange("p (a b) -> p a b", b=NG)
                    dCv = dC[:].rearrange("p (a b) -> p a b", b=NG)
                    nc.vector.tensor_tensor(dAv, jb, ja, op=OP.is_equal)
                    nc.vector.tensor_tensor(dBv, bb, ba, op=OP.is_gt)
                    nc.vector.tensor_tensor(dCv, bb, ba, op=OP.is_equal)
                    ltcv = LTC[:].rearrange("p (a b) -> p a b", b=NG)
                    nc.vector.tensor_tensor(dCv, dCv, ltcv, op=OP.mult)
                    nc.vector.tensor_tensor(dBv, dBv, dCv, op=OP.max)
                    nc.vector.tensor_tensor(dAv, dAv, dBv, op=OP.mult)
                    KILL = sp.tile([P, NG], u16, tag="kill")
                    nc.vector.tensor_reduce(KILL[:], dAv, axis=AX.X, op=OP.max)
                    JEFF = sp.tile([P, NG], fp32, tag="jeff")
                    nc.vector.select(JEFF[:], KILL[:], MINUS1[:], JF[:])

                    # ---- halves + int16 indices ----
                    GEH = sp.tile([P, NG], u16, tag="geh")
                    nc.vector.tensor_scalar(GEH[:], JEFF[:], 1024.0, None,
                                            op0=OP.is_ge)
                    JAf = sp.tile([P, NG], fp32, tag="jaf")
                    JBm = sp.tile([P, NG], fp32, tag="jbm")
                    JBf = sp.tile([P, NG], fp32, tag="jbf")
                    nc.vector.select(JAf[:], GEH[:], MINUS1[:], JEFF[:])
                    nc.vector.tensor_scalar(JBm[:], JEFF[:], -1024.0, None,
                                            op0=OP.add)
                    nc.vector.select(JBf[:], GEH[:], JBm[:], MINUS1[:])
                    JA16 = sp.tile([P, NG], i16, tag="ja16")
                    JB16 = sp.tile([P, NG], i16, tag="jb16")
                    nc.vector.tensor_copy(JA16[:], JAf[:])
                    nc.vector.tensor_copy(JB16[:], JBf[:])

                    # ---- bid bit-planes ----
                    bbits = BIDF[:].bitcast(u16).rearrange(
                        "p (k two) -> p k two", two=2)
                    BLO = sp.tile([P, NG], u16, tag="blo")
                    BHI = sp.tile([P, NG], u16, tag="bhi")
                    nc.vector.tensor_copy(BLO[:], bbits[:, :, 0])
                    nc.vector.tensor_copy(BHI[:], bbits[:, :, 1])

                    # ---- dense scatter + partition max ----
                    MHI = dp.tile([P, N], u16, tag="mhi")
                    MLO = dp.tile([P, N], u16, tag="mlo")
                    for half, idxs in ((0, JA16), (1, JB16)):
                        nc.gpsimd.local_scatter(
                            MHI[:, 1024 * half:1024 * (half + 1)], BHI[:],
                            idxs[:], channels=P, num_elems=1024, num_idxs=NG)
                        nc.gpsimd.local_scatter(
                            MLO[:, 1024 * half:1024 * (half + 1)], BLO[:],
                            idxs[:], channels=P, num_elems=1024, num_idxs=NG)
                    CHI = dp.tile([P, N], u16, tag="chi")
                    nc.gpsimd.partition_all_reduce(CHI[:], MHI[:], channels=P,
                                                   reduce_op=bass_isa.ReduceOp.max)
                    EQD = dp.tile([P, N], u16, tag="eqd")
                    nc.vector.tensor_tensor(EQD[:], MHI[:], CHI[:], op=OP.is_equal)
                    SLO = dp.tile([P, N], u16, tag="mhi")
                    nc.vector.tensor_tensor(SLO[:], MLO[:], EQD[:], op=OP.mult)
                    CLO = dp.tile([P, N], u16, tag="mlo")
                    nc.gpsimd.partition_all_reduce(CLO[:], SLO[:], channels=P,
                                                   reduce_op=bass_isa.ReduceOp.max)

                    # ---- combine planes bit-exactly (u16 half-copies), update price ----
                    PB32 = dp.tile([P, N], i32, tag="pb32")
                    pnew16 = PB32[:].bitcast(u16).rearrange(
                        "p (n two) -> p n two", two=2)
                    nc.vector.tensor_copy(pnew16[:, :, 0], CLO[:])
                    nc.vector.tensor_copy(pnew16[:, :, 1], CHI[:])
                    GTZ = dp.tile([P, N], u16, tag="eqd")
                    nc.vector.tensor_scalar(GTZ[:], CHI[:], 0, None, op0=OP.is_gt)
                    nc.vector.copy_predicated(PBC[:], GTZ[:],
                                              PB32[:].bitcast(fp32))

                    if DBG and s == 0:
                        nc.sync.dma_start(dbgit_d.ap()[it, 2], PBC[:])
                        nc.sync.dma_start(dbgit_d.ap()[it, 3][:, 0:NG], JEFF[:])
                        nc.sync.dma_start(dbgit_d.ap()[it, 3][:, 32:32 + NG],
                                          BIDF[:])
                    if DBG and s == 0 and it == 0:
                        for g in range(NG):
                            nc.sync.dma_start(dbgv_d.ap()[g],
                                              V0[:, g * N:(g + 1) * N])
                        nc.sync.dma_start(dbg_d.ap()[0], V0[:, 0:N])
                        nc.sync.dma_start(dbg_d.ap()[1], PBC[:])
                        nc.sync.dma_start(dbg_d.ap()[2][:, 0:NG * 8], TOP8[:])
                        f3 = dbg_d.ap()[3]
                        nc.sync.dma_start(f3[:, 256:256 + NG], JF[:])
                        nc.sync.dma_start(f3[:, 288:288 + NG], BIDF[:])
                        nc.sync.dma_start(f3[:, 320:320 + NG], JEFF[:])
                        nc.sync.dma_start(f3[:, 352:352 + NG], KILL[:])
                        nc.sync.dma_start(dbg_d.ap()[7], PB32[:].bitcast(fp32))
                        u = dbgu_d.ap()
                        nc.sync.dma_start(u[0], MHI[:])
                        nc.sync.dma_start(u[1], CHI[:])
                        nc.sync.dma_start(u[2], CLO[:])
                        nc.sync.dma_start(u[3][:, 0:NG * 8], IDX8[:])
                        nc.sync.dma_start(u[3][:, 384:384 + NG], BHI[:])
                        nc.sync.dma_start(u[3][:, 416:416 + NG], BLO[:])

                # ================= output =================
                # x0 coord broadcasts (reuse dead iteration-stage slots)
                x0bcs = []
                for c, tg in zip(range(3), ("vps", "vps", "pb32")):
                    X0C = dp.tile([P, N], fp32, tag=tg,
                                  bufs=3 if tg == "vps" else None)
                    for tcol in range(4):
                        pb = psA.tile([P, 512], fp32, tag="ps")
                        nc.tensor.matmul(pb[:], ECS[32:35, c * P:(c + 1) * P],
                                         X04[0:3, 512 * tcol:512 * (tcol + 1)])
                        nc.scalar.activation(
                            X0C[:, 512 * tcol:512 * (tcol + 1)],
                            pb[:], AF.Identity, bias=0.0, scale=-0.5)
                    x0bcs.append(X0C)
                xa = sp.tile([P, NG * D], fp32, tag="xa")
                EQ = dp.tile([P, N], u16, tag="chi")
                MM = dp.tile([P, N], fp32, tag="pbc")
                IOTAU = dp.tile([P, N], u16, tag="eqd")
                nc.gpsimd.iota(IOTAU[:], pattern=[[1, N]], base=0,
                               channel_multiplier=0)
                for g in range(NG):
                    nc.vector.tensor_scalar(EQ[:], IOTAU[:], JF[:, g:g + 1],
                                            None, op0=OP.is_equal)
                    for c in range(3):
                        # one-hot mask -> sum has a single nonzero term (exact)
                        nc.vector.affine_mul_reduce(
                            out=MM[:],
                            accum_out=xa[:, g * D + c:g * D + c + 1],
                            in0=EQ[:], in1=x0bcs[c][:], scale=1.0, bias=0.0)

                tb1 = sp.tile([1, 1], fp32, tag="tb1")
                nc.sync.dma_start(tb1[:], tv_d.ap()[s].unsqueeze(0))
                TB = sp.tile([P, 1], fp32, tag="tbb")
                nc.gpsimd.partition_broadcast(TB[:], tb1[:], channels=P)
                OMT = sp.tile([P, 1], fp32, tag="omt")
                nc.vector.tensor_scalar(OMT[:], TB[:], -1.0, 1.0,
                                        op0=OP.mult, op1=OP.add)
                XT = sp.tile([P, NG * D], fp32, tag="xt")
                NTt = sp.tile([P, NG * D], fp32, tag="ntt")
                VV = sp.tile([P, NG * D], fp32, tag="vv")
                nc.vector.tensor_scalar(XT[:], xa[:], OMT[:], None,
                                        op0=OP.mult)
                nc.vector.tensor_scalar(NTt[:], nR[:], TB[:], None,
                                        op0=OP.mult)
                nc.vector.tensor_tensor(XT[:], XT[:], NTt[:], op=OP.add)
                nc.vector.tensor_tensor(VV[:], nR[:], xa[:], op=OP.subtract)
                nc.sync.dma_start(out_d.ap()[s, 0], XT[:])
                nc.sync.dma_start(out_d.ap()[s, 1], VV[:])

    nc.compile()
    return nc


_NC_CACHE = None


def _get_nc():
    global _NC_CACHE
    if _NC_CACHE is None:
        _NC_CACHE = _build_program()
    return _NC_CACHE


def _host_prep(cloud, noise, t):
    """Build per-core input maps."""
    B = cloud.shape[0]
    ltc = np.zeros((P, NG, NG), np.uint16)
    for g in range(NG):
        ltc[:, g, :g] = 1
    ltc = ltc.reshape(P, NG * NG).astype(np.uint16)
    ecs = np.zeros((35, 3 * P), np.float32)
    for c in range(3):
        ecs[32 + c, c * P:(c + 1) * P] = 1.0
    onesrow = np.ones((1, N), np.float32)
    in_maps = []
    for c in range(NCORES):
        sidx = [c * SPC + k for k in range(SPC)]
        noiseT = np.stack([noise[s].T for s in sidx]).astype(np.float32)
        cloudT = np.stack([cloud[s].T for s in sidx]).astype(np.float32)
        cloudR = np.stack([
            cloud[s].reshape(NG, P, D).transpose(1, 0, 2).reshape(P, NG * D)
            for s in sidx]).astype(np.float32)
        noiseR = np.stack([
            noise[s].reshape(NG, P, D).transpose(1, 0, 2).reshape(P, NG * D)
            for s in sidx]).astype(np.float32)
        tv = np.array([[t[s]] for s in sidx], np.float32)
        in_maps.append({
            "noiseT": np.ascontiguousarray(noiseT),
            "cloudT": np.ascontiguousarray(cloudT),
            "cloudR": np.ascontiguousarray(cloudR),
            "noiseR": np.ascontiguousarray(noiseR),
            "tv": tv, "ltc": ltc, "ecs": ecs, "onesrow": onesrow,
        })
    return in_maps


def _host_post(results, B):
    out = np.zeros((2, B, N, D), np.float32)
    for c in range(NCORES):
        o = results[c]["out"]  # [SPC, 2, P, NG*D]
        for k in range(SPC):
            s = c * SPC + k
            for which in range(2):
                arr = o[k, which].reshape(P, NG, D).transpose(1, 0, 2)
                out[which, s] = arr.reshape(N, D)
    return out


def kernel(cloud, noise, t):
    from concourse import bass_utils
    cloud = np.asarray(cloud, np.float32)
    noise = np.asarray(noise, np.float32)
    t = np.asarray(t, np.float32)
    nc = _get_nc()
    in_maps = _host_prep(cloud, noise, t)
    res = bass_utils.run_bass_kernel_spmd(nc, in_maps,
                                          core_ids=list(range(NCORES)))
    return _host_post(res.results, cloud.shape[0])


if __name__ == "__main__":
    import sys
    sys.path.insert(0, "/root/problem")
    d = np.load("/root/problem/ref_io.npz")
    out = kernel(d["cloud"], d["noise"], d["t"])
    for name in ("out_cpu", "out_axon", "mine"):
        ref = d[name]
        rv = float(((ref - out) ** 2).mean() / ((ref ** 2).mean() + 1e-8))
        print(f"resid_var vs {name}: {rv:.3e}")
